# revision 7
# baseline (speedup 1.0000x reference)
"""Trainium2 Bass kernel for DCEModulatedResBlock.

Strategy (8 NeuronCores, data-parallel over batch B=16 -> 2 images/core):
  - x kept resident in SBUF (f32r), channels on partitions, rows padded to
    129 elements with one shared zero column (kills 3x3-conv wraparound).
  - Modulation (dce FFN x spatial stats) folded into conv1/sc WEIGHTS per
    image (xm = x * mod[c] is never materialized: W'[ci,:] = W[ci,:]*mod[ci]).
  - conv1 (3x3) as 9 accumulated float32r matmuls per 4-row chunk.
  - BatchNorm batch stats via two tiny AllReduces across the 8 cores
    (sum / sumsq per channel), computed with bn_stats/bn_aggr.
  - y1 / y2 share one bf16 SBUF buffer (y2 overwrites y1 chunk-by-chunk);
    sc-branch conv (1x1) is recomputed in phase C from resident x.
"""

import sys

sys.path.insert(0, "/opt/trn_rl_repo")

import numpy as np
from contextlib import ExitStack

import concourse.bass as bass
import concourse.bacc as bacc
import concourse.tile as tile
from concourse import mybir
from concourse.bass_utils import run_bass_kernel_spmd

f32 = mybir.dt.float32
f32r = mybir.dt.float32r
bf16 = mybir.dt.bfloat16
AF = mybir.ActivationFunctionType
ALU = mybir.AluOpType

N_CORES = 8
BL = 2          # images per core
C = 128
H = W = 128
HW = H * W      # 16384
WP = W + 1      # padded row stride (col 0 is the shared zero pad)
XLEN = H * WP + 1   # + trailing zero so row 127 dw=+1 stays in range
CH = 512        # chunk size (pixels) = 4 rows
RPC = CH // W   # rows per chunk
NCH = HW // CH  # 32 chunks per image
NLOC = float(BL * HW)     # local pixel count per channel
NTOT = float(16 * HW)     # global pixel count per channel
EPS = 1e-5
INV_SQRT2 = 0.7071067811865476

_CACHE = {}


def fap(t, offset, pairs):
    """AP over tile t's free dim: element `offset`, free pattern `pairs`."""
    base = t[:, 0:1]
    return bass.AP(tensor=base.tensor, offset=base.offset + offset,
                   ap=[base.ap[0]] + [list(p) for p in pairs])


def _gelu(nc, pool, out_ap, in_ap, bias_ap, p, n):
    """out = gelu_exact(in + bias) onto out_ap ([p, n]). in_ap may be PSUM."""
    t = pool.tile([p, n], f32, tag="gelu_t")
    nc.scalar.activation(t, in_ap, AF.Identity, bias=bias_ap, scale=1.0)
    e = pool.tile([p, n], f32, tag="gelu_e")
    nc.scalar.activation(e, t, AF.Erf, bias=0.0, scale=INV_SQRT2)
    ep = pool.tile([p, n], f32, tag="gelu_ep")
    nc.vector.tensor_scalar(ep, e, 0.5, 0.5, ALU.mult, ALU.add)
    nc.vector.tensor_mul(out_ap, t, ep)


def build():
    nc = bacc.Bacc("TRN2", target_bir_lowering=False, debug=False,
                   num_devices=N_CORES)

    x_d = nc.dram_tensor("x", [BL, C, XLEN], f32r, kind="ExternalInput")
    dce_d = nc.dram_tensor("dce_rhs", [C, 100, BL], f32, kind="ExternalInput")
    wd1_d = nc.dram_tensor("w_dce1", [100, C, C], f32, kind="ExternalInput")
    bd1_d = nc.dram_tensor("b_dce1", [C, 1], f32, kind="ExternalInput")
    wd2_d = nc.dram_tensor("w_dce2", [C, C], f32, kind="ExternalInput")
    bd2_d = nc.dram_tensor("b_dce2", [C, 1], f32, kind="ExternalInput")
    wsh_d = nc.dram_tensor("w_sh", [C, 64], f32, kind="ExternalInput")
    bsh_d = nc.dram_tensor("b_sh", [64, 1], f32, kind="ExternalInput")
    wex_d = nc.dram_tensor("w_ex", [64, C], f32, kind="ExternalInput")
    bex_d = nc.dram_tensor("b_ex", [C, 1], f32, kind="ExternalInput")
    wcoef_d = nc.dram_tensor("wcoef", [C, 9], f32, kind="ExternalInput")
    w1t_d = nc.dram_tensor("w1t", [C, 9, C], f32, kind="ExternalInput")
    w2_d = nc.dram_tensor("w2", [C, C], f32r, kind="ExternalInput")
    wsc_d = nc.dram_tensor("wsc", [C, C], f32, kind="ExternalInput")
    bn_d = {nm: nc.dram_tensor(nm, [C, 1], f32, kind="ExternalInput")
            for nm in ["bn1_g", "bn1_b", "bn2_g", "bn2_b", "bnsc_g", "bnsc_b"]}
    out_d = nc.dram_tensor("out", [BL, C, HW], f32, kind="ExternalOutput")

    with tile.TileContext(nc) as tc, ExitStack() as ctx:
        const = ctx.enter_context(tc.tile_pool(name="const", bufs=1))
        yyp = ctx.enter_context(tc.tile_pool(name="yyp", bufs=1))
        statp = ctx.enter_context(tc.tile_pool(name="statp", bufs=1))
        xpool = ctx.enter_context(tc.tile_pool(name="xpool", bufs=1))
        dram = ctx.enter_context(tc.tile_pool(name="dram", bufs=1, space="DRAM"))
        ps_c1 = ctx.enter_context(tc.tile_pool(name="ps_c1", bufs=3, space="PSUM"))
        ps_sc = ctx.enter_context(tc.tile_pool(name="ps_sc", bufs=2, space="PSUM"))
        ps_sm = ctx.enter_context(tc.tile_pool(name="ps_sm", bufs=1, space="PSUM"))

        # ---------- constant loads ----------
        def cvec(dram_t, p=C, n=1, dt=f32, tag=None):
            t = const.tile([p, n], dt, tag=tag or dram_t.name + "_sb",
                           name=(tag or dram_t.name + "_sb"))
            nc.sync.dma_start(out=t, in_=dram_t.ap())
            return t

        bd1 = cvec(bd1_d)
        bd2 = cvec(bd2_d)
        bsh = cvec(bsh_d, p=64)
        bex = cvec(bex_d)
        wcoef = cvec(wcoef_d, n=9)
        bn_sb = {nm: cvec(bn_d[nm]) for nm in bn_d}
        w2_sb = const.tile([C, C], f32r, tag="w2_sb")
        nc.sync.dma_start(out=w2_sb, in_=w2_d.ap())
        wsh = const.tile([C, 64], f32, tag="wsh_sb")
        nc.sync.dma_start(out=wsh, in_=wsh_d.ap())
        wex = const.tile([64, C], f32, tag="wex_sb")
        nc.sync.dma_start(out=wex, in_=wex_d.ap())
        eps_t = const.tile([C, 1], f32, tag="eps_t")
        nc.vector.memset(eps_t, EPS)
        mod = const.tile([C, BL], f32, tag="mod")     # per-image channel scales
        spat = const.tile([C, BL], f32, tag="spat")
        dcef = const.tile([C, BL], f32, tag="dcef")

        # persistent y (y1 then y2) bf16 chunk tiles
        yy = [[yyp.tile([C, CH], bf16, tag=f"yy_{b}_{k}", name=f"yy_{b}_{k}")
               for k in range(NCH)] for b in range(BL)]
        # stats strips
        st_c1 = statp.tile([C, BL * NCH, 6], f32, tag="st_c1")
        st_sc = statp.tile([C, BL * NCH, 6], f32, tag="st_sc")
        ar1_in = statp.tile([C, 4], f32, tag="ar1_in")
        ar1_out = statp.tile([C, 4], f32, tag="ar1_out")
        ar2_in = statp.tile([C, 2], f32, tag="ar2_in")
        ar2_out = statp.tile([C, 2], f32, tag="ar2_out")
        a1 = statp.tile([C, 1], f32, tag="a1")
        d1 = statp.tile([C, 1], f32, tag="d1")
        asc = statp.tile([C, 1], f32, tag="asc")
        dsc = statp.tile([C, 1], f32, tag="dsc")
        a2 = statp.tile([C, 1], f32, tag="a2")
        dd = statp.tile([C, 1], f32, tag="dd")   # d2 + dsc

        # resident x (both images), padded-row layout
        x_sb = [xpool.tile([C, XLEN], f32r, tag=f"x_{b}", name=f"x_{b}")
                for b in range(BL)]

        # ---------- phase 0: dce FFN (both images, N=2) ----------
        with tc.tile_pool(name="p0", bufs=4) as p0:
            dce_sb = p0.tile([C, 100, BL], f32, tag="dce_sb")
            nc.sync.dma_start(out=dce_sb, in_=dce_d.ap())
            wd2 = p0.tile([C, C], f32, tag="wd2_sb")
            nc.sync.dma_start(out=wd2, in_=wd2_d.ap())
            h0 = ps_sm.tile([C, BL], f32, tag="h0")
            for l in range(100):
                w1l = p0.tile([C, C], f32, tag="wd1_stream")
                nc.sync.dma_start(out=w1l, in_=wd1_d.ap()[l])
                nc.tensor.matmul(h0, w1l, dce_sb[:, l, :],
                                 start=(l == 0), stop=(l == 99))
            hact = p0.tile([C, BL], f32, tag="hact")
            _gelu(nc, p0, hact, h0, bd1, C, BL)
            dps = ps_sm.tile([C, BL], f32, tag="mm")
            nc.tensor.matmul(dps, wd2, hact, start=True, stop=True)
            nc.scalar.activation(dcef, dps, AF.Identity, bias=bd2, scale=1.0)

        # ---------- phases 1+2+A per image ----------
        with tc.tile_pool(name="pA", bufs=1) as pA, \
             tc.tile_pool(name="pAs", bufs=2) as pAs:
            w1s = pA.tile([C, 9, C], f32r, tag="w1s")       # scaled conv1 taps
            wscs = pA.tile([C, C], f32r, tag="wscs")        # scaled sc weights

            for b in range(BL):
                xt = x_sb[b]
                # load padded x image (8 parallel contiguous DMAs)
                bounds = [round(XLEN * j / 8) for j in range(9)]
                for j in range(8):
                    nc.sync.dma_start(
                        out=xt[:, bounds[j]:bounds[j + 1]],
                        in_=x_d.ap()[b, :, bounds[j]:bounds[j + 1]])

                # spatial sums -> spat[:, b]
                R = pAs.tile([C, H], f32, tag="R", bufs=1)
                nc.vector.reduce_sum(out=R, in_=fap(xt, 1, [[WP, H], [1, W]]),
                                     axis=mybir.AxisListType.X)
                svec = pAs.tile([C, 9], f32, tag="svec")
                nc.vector.reduce_sum(out=svec[:, 0:1], in_=R,
                                     axis=mybir.AxisListType.X)          # T
                nc.vector.tensor_copy(out=svec[:, 1:2], in_=R[:, H - 1:H])  # R127
                nc.vector.tensor_copy(out=svec[:, 2:3], in_=R[:, 0:1])      # R0
                nc.vector.reduce_sum(out=svec[:, 3:4],
                                     in_=fap(xt, W, [[WP, H]]),
                                     axis=mybir.AxisListType.X)          # C127
                nc.vector.reduce_sum(out=svec[:, 4:5],
                                     in_=fap(xt, 1, [[WP, H]]),
                                     axis=mybir.AxisListType.X)          # C0
                nc.vector.tensor_copy(out=svec[:, 5:6],
                                      in_=fap(xt, (H - 1) * WP + W, [[1, 1]]))
                nc.vector.tensor_copy(out=svec[:, 6:7],
                                      in_=fap(xt, (H - 1) * WP + 1, [[1, 1]]))
                nc.vector.tensor_copy(out=svec[:, 7:8],
                                      in_=fap(xt, W, [[1, 1]]))
                nc.vector.tensor_copy(out=svec[:, 8:9],
                                      in_=fap(xt, 1, [[1, 1]]))
                sprod = pAs.tile([C, 9], f32, tag="sprod")
                nc.vector.tensor_mul(sprod, svec, wcoef)
                nc.vector.reduce_sum(out=spat[:, b:b + 1], in_=sprod,
                                     axis=mybir.AxisListType.X)

                # modulation chain -> mod[:, b]  (plain fp32 matmuls, N=1)
                m_t = pAs.tile([C, 1], f32, tag="m_t")
                nc.vector.tensor_mul(m_t, dcef[:, b:b + 1], spat[:, b:b + 1])
                shp = ps_sm.tile([64, 1], f32, tag="mm")
                nc.tensor.matmul(shp, wsh, m_t, start=True, stop=True)
                sha = pAs.tile([64, 1], f32, tag="sha")
                _gelu(nc, pAs, sha, shp, bsh, 64, 1)
                exp_ = ps_sm.tile([C, 1], f32, tag="mm")
                nc.tensor.matmul(exp_, wex, sha, start=True, stop=True)
                nc.scalar.activation(mod[:, b:b + 1], exp_, AF.Sigmoid,
                                     bias=bex, scale=1.0)

                # scale conv weights by mod[:, b] (per input channel)
                for t in range(9):
                    wst = pAs.tile([C, C], f32, tag="wst", bufs=1)
                    nc.sync.dma_start(out=wst, in_=w1t_d.ap()[:, t, :])
                    nc.vector.tensor_scalar_mul(w1s[:, t, :], wst, mod[:, b:b + 1])
                wscst = pAs.tile([C, C], f32, tag="wst", bufs=1)
                nc.sync.dma_start(out=wscst, in_=wsc_d.ap())
                nc.vector.tensor_scalar_mul(wscs, wscst, mod[:, b:b + 1])

                # conv1 + sc over 32 chunks
                for k in range(NCH):
                    r0 = k * RPC
                    ps = ps_c1.tile([C, CH], f32, tag="c1")
                    first = True
                    for t in [4, 0, 1, 2, 3, 5, 6, 7, 8]:
                        dh, dw = t // 3 - 1, t % 3 - 1
                        i0 = max(0, -(r0 + dh))
                        i1 = min(RPC, H - (r0 + dh))
                        rhs = fap(xt, (r0 + i0 + dh) * WP + 1 + dw,
                                  [[WP, i1 - i0], [1, W]])
                        nc.tensor.matmul(ps[:, i0 * W:i1 * W], w1s[:, t, :], rhs,
                                         start=first, stop=(t == 8))
                        first = False
                    # sc 1x1 conv (stats only in phase A)
                    ps2 = ps_sc.tile([C, CH], f32, tag="sc")
                    nc.tensor.matmul(ps2, wscs,
                                     fap(xt, r0 * WP + 1, [[WP, RPC], [1, W]]),
                                     start=True, stop=True)
                    # evacuate y1 (bf16) + stats
                    nc.scalar.copy(yy[b][k], ps)
                    nc.vector.bn_stats(out=st_c1[:, b * NCH + k, :], in_=ps)
                    nc.vector.bn_stats(out=st_sc[:, b * NCH + k, :], in_=ps2)

        # ---------- AllReduce 1 (bn1 + bnsc stats) ----------
        def pack_stats(strip, ar_tile, off):
            mv = statp.tile([C, 2], f32, tag=f"mv_{off}", name=f"mv_{off}")
            nc.vector.bn_aggr(out=mv, in_=strip)
            nc.vector.tensor_scalar_mul(ar_tile[:, off:off + 1], mv[:, 0:1], NLOC)
            sq = statp.tile([C, 1], f32, tag=f"sq_{off}", name=f"sq_{off}")
            nc.vector.tensor_mul(sq, mv[:, 0:1], mv[:, 0:1])
            nc.vector.tensor_add(sq, mv[:, 1:2], sq)
            nc.vector.tensor_scalar_mul(ar_tile[:, off + 1:off + 2], sq, NLOC)

        pack_stats(st_c1, ar1_in, 0)
        pack_stats(st_sc, ar1_in, 2)
        ar1_di = dram.tile([C, 4], f32, tag="ar1_di")
        ar1_do = dram.tile([C, 4], f32, tag="ar1_do")
        nc.sync.dma_start(out=ar1_di, in_=ar1_in)
        nc.gpsimd.collective_compute(
            "AllReduce", ALU.add, replica_groups=[list(range(N_CORES))],
            ins=[ar1_di.opt()], outs=[ar1_do.opt()])
        nc.sync.dma_start(out=ar1_out, in_=ar1_do)

        def derive_affine(ar_tile, off, g_sb, b_sb, a_t, d_t, pool):
            gm = pool.tile([C, 1], f32, tag=f"gm_{off}", name=f"gm_{off}", bufs=1)
            nc.vector.tensor_scalar_mul(gm, ar_tile[:, off:off + 1], 1.0 / NTOT)
            vg = pool.tile([C, 1], f32, tag=f"vg_{off}", name=f"vg_{off}", bufs=1)
            nc.vector.tensor_scalar_mul(vg, ar_tile[:, off + 1:off + 2], 1.0 / NTOT)
            msq = pool.tile([C, 1], f32, tag=f"msq_{off}", name=f"msq_{off}",
                            bufs=1)
            nc.vector.tensor_mul(msq, gm, gm)
            nc.vector.tensor_sub(vg, vg, msq)
            sd = pool.tile([C, 1], f32, tag=f"sd_{off}", name=f"sd_{off}", bufs=1)
            nc.scalar.activation(sd, vg, AF.Sqrt, bias=eps_t, scale=1.0)
            rstd = pool.tile([C, 1], f32, tag=f"rstd_{off}", name=f"rstd_{off}",
                             bufs=1)
            nc.vector.reciprocal(rstd, sd)
            nc.vector.tensor_mul(a_t, g_sb, rstd)
            tmp = pool.tile([C, 1], f32, tag=f"tmp_{off}", name=f"tmp_{off}",
                            bufs=1)
            nc.vector.tensor_mul(tmp, a_t, gm)
            nc.vector.tensor_sub(d_t, b_sb, tmp)

        derive_affine(ar1_out, 0, bn_sb["bn1_g"], bn_sb["bn1_b"], a1, d1, statp)
        derive_affine(ar1_out, 2, bn_sb["bnsc_g"], bn_sb["bnsc_b"], asc, dsc,
                      statp)

        # ---------- phase B: z = silu(bn1(y1)); y2 = conv2(z) ----------
        with tc.tile_pool(name="pB", bufs=3) as pB:
            st_y2 = pB.tile([C, BL * NCH, 6], f32, tag="st_y2", bufs=1)
            for b in range(BL):
                for k in range(NCH):
                    z = pB.tile([C, CH], f32r, tag="z", bufs=2)
                    nc.scalar.activation(z, yy[b][k], AF.Silu, bias=d1, scale=a1)
                    ps = ps_c1.tile([C, CH], f32, tag="c1")
                    nc.tensor.matmul(ps, w2_sb, z, start=True, stop=True)
                    nc.vector.bn_stats(out=st_y2[:, b * NCH + k, :], in_=ps)
                    nc.scalar.copy(yy[b][k], ps)   # overwrite y1 with y2

            # ---------- AllReduce 2 (bn2 stats) ----------
            mv = pB.tile([C, 2], f32, tag="mv_y2", bufs=1)
            nc.vector.bn_aggr(out=mv, in_=st_y2)
            nc.vector.tensor_scalar_mul(ar2_in[:, 0:1], mv[:, 0:1], NLOC)
            sq = pB.tile([C, 1], f32, tag="sq_y2", bufs=1)
            nc.vector.tensor_mul(sq, mv[:, 0:1], mv[:, 0:1])
            nc.vector.tensor_add(sq, mv[:, 1:2], sq)
            nc.vector.tensor_scalar_mul(ar2_in[:, 1:2], sq, NLOC)
            ar2_di = dram.tile([C, 2], f32, tag="ar2_di")
            ar2_do = dram.tile([C, 2], f32, tag="ar2_do")
            nc.sync.dma_start(out=ar2_di, in_=ar2_in)
            nc.gpsimd.collective_compute(
                "AllReduce", ALU.add, replica_groups=[list(range(N_CORES))],
                ins=[ar2_di.opt()], outs=[ar2_do.opt()])
            nc.sync.dma_start(out=ar2_out, in_=ar2_do)
            d2 = pB.tile([C, 1], f32, tag="d2", bufs=1)
            derive_affine(ar2_out, 0, bn_sb["bn2_g"], bn_sb["bn2_b"], a2, d2, pB)
            nc.vector.tensor_add(dd, d2, dsc)

        # ---------- phase C: out = silu(bn2(y2) + bnsc(sc(x))) ----------
        with tc.tile_pool(name="pC", bufs=2) as pC:
            wscs_c = [pC.tile([C, C], f32r, tag=f"wscs_c{b}", name=f"wscs_c{b}",
                              bufs=1) for b in range(BL)]
            for b in range(BL):
                wscst = pC.tile([C, C], f32, tag="wsc_st", bufs=1)
                nc.sync.dma_start(out=wscst, in_=wsc_d.ap())
                nc.vector.tensor_scalar_mul(wscs_c[b], wscst, mod[:, b:b + 1])
            for b in range(BL):
                xt = x_sb[b]
                for k in range(NCH):
                    r0 = k * RPC
                    ps2 = ps_sc.tile([C, CH], f32, tag="sc")
                    nc.tensor.matmul(ps2, wscs_c[b],
                                     fap(xt, r0 * WP + 1, [[WP, RPC], [1, W]]),
                                     start=True, stop=True)
                    v = pC.tile([C, CH], f32, tag="v", bufs=3)
                    nc.scalar.activation(v, ps2, AF.Identity, bias=dd, scale=asc)
                    nc.vector.scalar_tensor_tensor(
                        v, yy[b][k], a2, v, ALU.mult, ALU.add)
                    nc.scalar.activation(v, v, AF.Silu)
                    nc.sync.dma_start(
                        out=out_d.ap()[b, :, k * CH:(k + 1) * CH], in_=v)

    nc.finalize()
    return nc


def _get_nc():
    if "nc" not in _CACHE:
        _CACHE["nc"] = build()
    return _CACHE["nc"]


def kernel(x, dce_output, dw_conv, W_dce1, b_dce1, W_dce2, b_dce2,
           W_sh, b_sh, W_ex, b_ex, conv1_w, bn1_g, bn1_b,
           conv2_w, bn2_g, bn2_b, sc_w, bnsc_g, bnsc_b, _trace=False):
    nc = _get_nc()
    ac = np.ascontiguousarray
    col = lambda v: ac(np.asarray(v, np.float32).reshape(-1, 1))

    # host-side weight layout prep (tiny tensors)
    w1t = ac(np.asarray(conv1_w, np.float32).transpose(1, 2, 3, 0)
             .reshape(C, 9, C))                       # [ci, tap, co]
    w2 = ac(np.asarray(conv2_w, np.float32)[:, :, 0, 0].T)   # [ci, co]
    wsc = ac(np.asarray(sc_w, np.float32)[:, :, 0, 0].T)
    wd1 = ac(np.asarray(W_dce1, np.float32).reshape(100, C, C))
    dw9 = np.asarray(dw_conv, np.float32).reshape(C, 9)
    # wcoef columns: [sum(w), -w_top, -w_bot, -w_left, -w_right, w0, w2, w6, w8]
    # (signs and 1/HW folded)
    wcoef = np.stack([
        dw9.sum(1), -dw9[:, 0:3].sum(1), -dw9[:, 6:9].sum(1),
        -dw9[:, [0, 3, 6]].sum(1), -dw9[:, [2, 5, 8]].sum(1),
        dw9[:, 0], dw9[:, 2], dw9[:, 6], dw9[:, 8]], axis=1) / HW
    wcoef = ac(wcoef.astype(np.float32))

    shared = dict(
        w_dce1=wd1, b_dce1=col(b_dce1), w_dce2=ac(np.asarray(W_dce2, np.float32)),
        b_dce2=col(b_dce2), w_sh=ac(np.asarray(W_sh, np.float32)),
        b_sh=col(b_sh), w_ex=ac(np.asarray(W_ex, np.float32)), b_ex=col(b_ex),
        wcoef=wcoef, w1t=w1t, w2=w2, wsc=wsc,
        bn1_g=col(bn1_g), bn1_b=col(bn1_b), bn2_g=col(bn2_g), bn2_b=col(bn2_b),
        bnsc_g=col(bnsc_g), bnsc_b=col(bnsc_b))

    in_maps = []
    x = np.asarray(x, np.float32)
    dce = np.asarray(dce_output, np.float32)
    # host-side zero-padding of rows to stride WP (pad col 0 + trailing zero)
    xp = np.zeros((16, C, XLEN), np.float32)
    xp[:, :, :H * WP].reshape(16, C, H, WP)[:, :, :, 1:] = \
        x.reshape(16, C, H, W)
    for c in range(N_CORES):
        in_maps.append(dict(
            x=ac(xp[BL * c:BL * (c + 1)]),
            dce_rhs=ac(dce[BL * c:BL * (c + 1)].transpose(2, 1, 0)),
            **shared))

    res = run_bass_kernel_spmd(nc, in_maps, core_ids=list(range(N_CORES)),
                               trace=_trace)
    out = np.empty((16, C, H, W), np.float32)
    for c in range(N_CORES):
        out[BL * c:BL * (c + 1)] = res.results[c]["out"].reshape(BL, C, H, W)
    if _trace:
        _CACHE["last_results"] = res
    return out
